# revision 1
# baseline (speedup 1.0000x reference)
"""Trainium2 Bass kernel for causal+padded multi-head attention.

Problem: B=2, N=2048, D=1024, H=16 heads (DK=64), fp32 I/O.
  out = softmax(mask(x Wq^T (x Wk^T)^T) / sqrt(DK)) (x Wv^T) Wout^T + b_out

Sharding (8 cores): core c handles batch b=c//4 and heads [4*(c%4), 4*(c%4)+4).
Each core computes a partial output [N, D] (its 4 heads' contribution through
the output projection); the host sums the 4 partials per batch and adds b_out.

On-device layout (per core):
  xT   [1024, 2048]  (host-pretransposed x[b])
  QT/KT stored transposed [dk, n] as head-pair tiles [128, 2048]
  V    stored natural as [128(keys), 16 blocks, 4 heads, 65] with a ones
       column appended (col 64) so P@V' also yields the softmax denominator.
  S^T  computed per (head-pair, q-tile 512, key-block 128) as [128, 2, 512]
       in PSUM: matmul(lhsT=KT slice [64,128], rhs=QT slice [64,512]); the
       two heads sit at base partitions 0/64 so their matmuls row-tile and
       run concurrently on the PE.
       Causal masking = additive -30000 on PSUM (DVE); padding mask is a
       per-key bias fused into the exp; one exp(0.125*s + bias) on ScalarE
       writes P^T straight to SBUF as bf16.
  ctx'^T [65, 512] accumulated in PSUM over key blocks:
       matmul(lhsT=V' [128,65], rhs=P^T [128,512]); PV matmuls are emitted
       one unit behind their exps so the in-order PE never waits on ScalarE
       (idle slivers would re-throttle the HAM clock gate to 1.2 GHz).
  Normalization: r = recip(rowsum) on the [1,512] denominator row (DVE),
       partition-broadcast to [64,512] (GpSimd), one DVE multiply;
       software-pipelined one unit behind the PV.
  Out projection: matmul(lhsT=ctxT [128,128], rhs=WoutT [128,512]) acc over
       the two head-pair chunks.

All matmul operands are bf16 (pre-rounded on host for the inputs; on-device
casts for intermediates); accumulation is fp32 in PSUM, and the softmax /
masking / normalization arithmetic is fp32. float32r was measured on this
hardware at 2 cycles/row with a serial (non-FWL) weight load that starves
the HAM activity monitor - bf16 is ~2.7x faster per matmul in practice.
"""

import math
import os

import numpy as np

B, N, D, H = 2, 2048, 1024, 16
DK = D // H  # 64
NCORES = 8
HEADS_PER_CORE = 4
QTILE = 512
KBLK = 128
NEG = -30000.0
NEGB = -3750.0  # pad bias applied after the 0.125 scale inside exp
SCALE = 1.0 / math.sqrt(float(DK))  # 0.125

# Set by run() when tracing is enabled (test.py reads this).
LAST_RESULTS = None


def _build_program(kb_max: int, jpad_min: int):
    import concourse.tile as tile
    from concourse import bacc, mybir

    F32 = mybir.dt.float32
    F32R = mybir.dt.float32r
    BF16 = mybir.dt.bfloat16
    EXP = mybir.ActivationFunctionType.Exp
    ADD = mybir.AluOpType.add

    nc = bacc.Bacc(None)

    xt_d = nc.dram_tensor("xt", [D, N], BF16, kind="ExternalInput")
    wq_d = nc.dram_tensor("wq", [D, 256], BF16, kind="ExternalInput")
    wk_d = nc.dram_tensor("wk", [D, 256], BF16, kind="ExternalInput")
    wv_d = nc.dram_tensor("wv", [D, 256], BF16, kind="ExternalInput")
    wout_d = nc.dram_tensor("wout", [256, D], BF16, kind="ExternalInput")
    padb_d = nc.dram_tensor("padbias", [128, 16], F32, kind="ExternalInput")
    trineg_d = nc.dram_tensor("trineg", [128, 896], F32, kind="ExternalInput")
    ones_d = nc.dram_tensor("ones65", [128, 64], BF16, kind="ExternalInput")
    out_d = nc.dram_tensor("out", [N, D], F32, kind="ExternalOutput")

    NB = N // KBLK  # 16 key/row blocks
    NQT = N // QTILE  # 4 q tiles

    with tile.TileContext(nc) as tc:
        with (
            tc.tile_pool(name="w", bufs=1) as w_pool,
            tc.tile_pool(name="big", bufs=1) as big_pool,
            tc.tile_pool(name="work", bufs=2) as work_pool,
            tc.tile_pool(name="ps_main", bufs=3, space="PSUM") as ps_main,
            tc.tile_pool(name="ps_ctx", bufs=1, space="PSUM") as ps_ctx,
        ):
            # ---- load inputs ----
            xt_cm = tc.tile_pool(name="xt", bufs=8)
            xt_pool = xt_cm.__enter__()
            wq_t = w_pool.tile([128, 8, 256], BF16, tag="wq")
            wk_t = w_pool.tile([128, 8, 256], BF16, tag="wk")
            wv_t = w_pool.tile([128, 8, 256], BF16, tag="wv")
            wo_t = w_pool.tile([128, 2, D], BF16, tag="wo")
            nc.sync.dma_start(wq_t[:], wq_d[:].rearrange("(e p) m -> p e m", p=128))
            nc.sync.dma_start(wk_t[:], wk_d[:].rearrange("(e p) m -> p e m", p=128))
            nc.sync.dma_start(wv_t[:], wv_d[:].rearrange("(e p) m -> p e m", p=128))
            padb_t = w_pool.tile([128, 16], F32, tag="padb")
            trineg_t = w_pool.tile([128, 896], F32, tag="trineg")
            nc.sync.dma_start(padb_t[:], padb_d[:])
            nc.sync.dma_start(trineg_t[:], trineg_d[:])
            xt = []
            for e in range(8):
                t = xt_pool.tile([128, N], BF16, tag="xt")
                nc.sync.dma_start(t[:], xt_d[e * 128:(e + 1) * 128, :])
                xt.append(t)
            # wout is not needed until phase D
            nc.sync.dma_start(wo_t[:], wout_d[:].rearrange("(c p) m -> p c m", p=128))

            # V' tile: [keys 128, key-block 16, head 4, 65]; col 64 <- ones
            v4 = big_pool.tile([128, NB, 4, 65], BF16, tag="v4")
            nc.sync.dma_start(
                v4[:, :, :, 64:65],
                ones_d[:].rearrange("p (b h o) -> p b h o", h=4, o=1),
            )

            qt_pair = [big_pool.tile([128, N], BF16, tag=f"qt{p}", name=f"qt{p}") for p in range(2)]
            kt_pair = [big_pool.tile([128, N], BF16, tag=f"kt{p}", name=f"kt{p}") for p in range(2)]
            ctx_pair = [big_pool.tile([128, N], BF16, tag=f"ctx{p}", name=f"ctx{p}") for p in range(2)]

            # ---- phase B: projections ----
            # QT/KT: [dk(128 = 2 heads), n] = (W.T chunk)^T @ xT
            for name, w_t, dst in (("q", wq_t, qt_pair), ("k", wk_t, kt_pair)):
                for pair in range(2):
                    for nq in range(NQT):
                        ps = ps_main.tile([128, 2, 512], F32, tag="blk", name="blk")[:, 0, :]
                        for e in range(8):
                            nc.tensor.matmul(
                                ps[:],
                                wq_t[:, e, pair * 128:(pair + 1) * 128]
                                if name == "q"
                                else wk_t[:, e, pair * 128:(pair + 1) * 128],
                                xt[e][:, nq * 512:(nq + 1) * 512],
                                start=(e == 0),
                                stop=(e == 7),
                            )
                        nc.vector.tensor_copy(
                            dst[pair][:, nq * 512:(nq + 1) * 512], ps[:]
                        )
            # V natural: [n-block, 4*64] = xT-chunk^T @ WvT-chunk
            for nb in range(NB):
                ps = ps_main.tile([128, 2, 512], F32, tag="blk", name="blk")[:, 0, 0:256]
                for e in range(8):
                    nc.tensor.matmul(
                        ps[:],
                        xt[e][:, nb * 128:(nb + 1) * 128],
                        wv_t[:, e, :],
                        start=(e == 0),
                        stop=(e == 7),
                    )
                nc.vector.tensor_copy(
                    v4[:, nb, :, 0:64],
                    ps[:].rearrange("p (h d) -> p h d", h=4),
                )

            xt_cm.__exit__(None, None, None)
            pt_cm = tc.tile_pool(name="pt", bufs=26)
            pt_pool = pt_cm.__enter__()

            # ---- phase C: attention, head pairs interleaved ----
            # A unit is (head-pair, q-tile). The two heads' S^T matmuls sit
            # at base partitions 0 / 64 (row groups 0-63 / 64-127), so they
            # execute concurrently on the PE and their weight loads overlap
            # the other head's matmul — no LDW bubble, HAM stays warm.
            # PV matmuls run one unit behind their exps so the in-order PE
            # never drains waiting on ScalarE.
            def emit_normalize(pair, hh, qt, ctx_ps):
                hp = slice(64 * hh, 64 * hh + 64)
                craw = work_pool.tile([65, 512], F32, tag="craw", name="craw")
                nc.scalar.copy(craw[:], ctx_ps[:])
                rrec = work_pool.tile([1, 512], F32, tag="rrec", name="rrec")
                nc.vector.reciprocal(rrec[:], craw[64:65, :])
                rbr = work_pool.tile([64, 512], F32, tag="rbr", name="rbr")
                nc.gpsimd.partition_broadcast(rbr[:], rrec[:])
                nc.vector.tensor_mul(
                    ctx_pair[pair][hp, qt * 512:(qt + 1) * 512],
                    craw[0:64, :],
                    rbr[:],
                )

            def emit_st_exp(pair, qt, nchunks, prev):
                """S^T + mask + exp for both heads, with the previous unit's
                PV matmuls riffled in (they are long-ready and fill the PE
                slots where S^T would stall on the exp pipeline). Returns
                PV descriptors."""
                if prev is None:
                    ppv = []
                else:
                    ppair, pqt, pn, ppv, pctx2 = prev

                def rif(k):
                    # emit previous-unit PV chunks up to index k
                    while ppv and ppv[0][0] <= k:
                        jj, ptt, poff = ppv.pop(0)
                        for hh in range(2):
                            nc.tensor.matmul(
                                pctx2[hh][:, poff:],
                                v4[:, jj, 2 * ppair + hh, :],
                                ptt[:, hh, poff:],
                                start=(jj == 0),
                                stop=(jj == pn - 1),
                                skip_group_check=True,
                            )

                pv = []
                for j in range(nchunks):
                    rif(j)
                    d = j - 4 * qt
                    # exact-causal column trim (keep matmul N >= 256)
                    off = 128 * d if d >= 1 else 0
                    st_ps = ps_main.tile([128, 2, 512], F32, tag="blk", name="blk")
                    for hh in range(2):
                        hp = slice(64 * hh, 64 * hh + 64)
                        nc.tensor.matmul(
                            st_ps[:, hh, off:],
                            kt_pair[pair][hp, j * 128:(j + 1) * 128],
                            qt_pair[pair][hp, qt * 512 + off:(qt + 1) * 512],
                            start=True,
                            stop=True,
                        )
                    if d >= 0:
                        # causal add -30000; with off = 128*d the masked
                        # triangle lies entirely in cols [off, off+128)
                        u0 = 384 - 128 * d + off
                        w = min(128, 512 - off)
                        for hh in range(2):
                            nc.vector.tensor_tensor(
                                st_ps[:, hh, off:off + w],
                                st_ps[:, hh, off:off + w],
                                trineg_t[:, u0:u0 + w],
                                ADD,
                            )
                    pt_t = pt_pool.tile([128, 2, 512], BF16, tag="pt")
                    kw = {}
                    if j >= jpad_min:  # per-key pad bias (same for both heads)
                        kw["bias"] = padb_t[:, j:j + 1]
                    nc.scalar.activation(
                        pt_t[:, :, off:], st_ps[:, :, off:], EXP, scale=SCALE, **kw
                    )
                    pv.append((j, pt_t, off))
                rif(10 ** 9)
                return pv

            def emit_pv(pair, qt, nchunks, pv, ctx2):
                for j, pt_t, off in pv:
                    for hh in range(2):
                        nc.tensor.matmul(
                            ctx2[hh][:, off:],
                            v4[:, j, 2 * pair + hh, :],
                            pt_t[:, hh, off:],
                            start=(j == 0),
                            stop=(j == nchunks - 1),
                            skip_group_check=True,
                        )

            units = [
                (pair, qt, min(4 * qt + 4, kb_max))
                for qt in range(NQT)
                for pair in range(2)
            ]
            done_norms = {q: 0 for q in range(NQT)}
            d_emitted = set()

            def emit_outproj(q):
                # output projection for the 4 n-blocks of q-tile q
                for nb in range(4 * q, 4 * q + 4):
                    osb = work_pool.tile([128, D], F32, tag="osb", name="osb")
                    for fc in range(2):
                        ps = ps_main.tile(
                            [128, 2, 512], F32, tag="blk", name="blk"
                        )[:, 0, :]
                        for pr2 in range(2):
                            nc.tensor.matmul(
                                ps[:],
                                ctx_pair[pr2][:, nb * 128:(nb + 1) * 128],
                                wo_t[:, pr2, fc * 512:(fc + 1) * 512],
                                start=(pr2 == 0),
                                stop=(pr2 == 1),
                            )
                        nc.scalar.copy(osb[:, fc * 512:(fc + 1) * 512], ps[:])
                    nc.sync.dma_start(out_d[nb * 128:(nb + 1) * 128, :], osb[:])

            def pop_norm():
                npair, nqt, nctx2 = norm_q.pop(0)
                for hh in range(2):
                    emit_normalize(npair, hh, nqt, nctx2[hh])
                done_norms[nqt] += 1
                if done_norms[nqt] == 2 and nqt not in d_emitted:
                    d_emitted.add(nqt)
                    emit_outproj(nqt)

            prev_pv = None  # (pair, qt, nchunks, pv_descs, ctx2)
            norm_q = []  # normalize one unit behind the PV
            for pair, qt, nchunks in units:
                pv = emit_st_exp(pair, qt, nchunks, prev_pv)
                if prev_pv is not None:
                    ppair, pqt, pn, ppv, pctx2 = prev_pv
                    norm_q.append((ppair, pqt, pctx2))
                if len(norm_q) > 1:
                    pop_norm()
                ctx2 = [
                    ps_ctx.tile([65, 512], F32, tag=f"ctx{hh}", name=f"ctx{hh}")
                    for hh in range(2)
                ]
                prev_pv = (pair, qt, nchunks, pv, ctx2)
            ppair, pqt, pn, ppv, pctx2 = prev_pv
            emit_pv(ppair, pqt, pn, ppv, pctx2)
            norm_q.append((ppair, pqt, pctx2))
            while norm_q:
                pop_norm()

            pt_cm.__exit__(None, None, None)

    nc.compile()
    return nc


_PROGRAM_CACHE = {}


def kernel(x, attention_mask, W_Q, W_K, W_V, W_out, b_out):
    global LAST_RESULTS
    from concourse.bass_utils import run_bass_kernel_spmd

    x = np.ascontiguousarray(x, dtype=np.float32)
    attention_mask = np.asarray(attention_mask)
    lengths = attention_mask.astype(np.int64).sum(axis=1)
    kb_max = int(math.ceil(lengths.max() / KBLK))
    jpad_min = int(lengths.min() // KBLK)

    key = (kb_max, jpad_min)
    if key not in _PROGRAM_CACHE:
        _PROGRAM_CACHE[key] = _build_program(kb_max, jpad_min)
    nc = _PROGRAM_CACHE[key]

    # host-side input prep (matmul operands pre-cast to bf16)
    import ml_dtypes
    BF = ml_dtypes.bfloat16
    xT = [np.ascontiguousarray(x[b].T.astype(BF)) for b in range(B)]
    wqT = np.ascontiguousarray(np.asarray(W_Q, dtype=np.float32).T.astype(BF))
    wkT = np.ascontiguousarray(np.asarray(W_K, dtype=np.float32).T.astype(BF))
    wvT = np.ascontiguousarray(np.asarray(W_V, dtype=np.float32).T.astype(BF))
    woT = np.ascontiguousarray(np.asarray(W_out, dtype=np.float32).T.astype(BF))
    # padbias[p, j] = 0 if key j*128+p is real else -30000
    padb = [
        np.ascontiguousarray(
            np.where(attention_mask[b].reshape(16, 128).T != 0, 0.0, NEGB)
        ).astype(np.float32)
        for b in range(B)
    ]
    # trineg[p, u] = NEG if u < p + 384 else 0; slice [384-128d : 896-128d]
    # gives the causal additive mask for a diagonal block with offset 128d.
    pp = np.arange(128)[:, None]
    uu = np.arange(896)[None, :]
    trineg = np.where(uu < pp + 384, NEG, 0.0).astype(np.float32)
    ones65 = np.ones((128, 64), dtype=BF)

    in_maps = []
    for c in range(NCORES):
        b, g = divmod(c, 4)
        sl = slice(g * 256, (g + 1) * 256)
        in_maps.append(
            {
                "xt": xT[b],
                "wq": np.ascontiguousarray(wqT[:, sl]),
                "wk": np.ascontiguousarray(wkT[:, sl]),
                "wv": np.ascontiguousarray(wvT[:, sl]),
                "wout": np.ascontiguousarray(woT[sl, :]),
                "padbias": padb[b],
                "trineg": trineg,
                "ones65": ones65,
            }
        )

    trace = bool(int(os.environ.get("KERNEL_TRACE", "0")))
    ncores_run = int(os.environ.get("KERNEL_NCORES", str(NCORES)))
    res = run_bass_kernel_spmd(
        nc,
        in_maps[:ncores_run],
        core_ids=list(range(ncores_run)),
        trace=trace,
        trace_cores=list(range(ncores_run)) if trace else None,
    )
    LAST_RESULTS = res

    out = np.zeros((B, N, D), dtype=np.float32)
    for c in range(len(res.results)):
        out[c // 4] += res.results[c]["out"]
    out += np.asarray(b_out, dtype=np.float32)[None, None, :]
    return out



# revision 53
# speedup vs baseline: 1.4191x; 1.4191x over previous
"""Trainium2 Bass kernel for causal+padded multi-head attention.

Problem: B=2, N=2048, D=1024, H=16 heads (DK=64), fp32 I/O.
  out = softmax(mask(x Wq^T (x Wk^T)^T) / sqrt(DK)) (x Wv^T) Wout^T + b_out

Sharding (8 cores): core c handles batch b=c//4 and heads [4*(c%4), 4*(c%4)+4).
Each core computes a partial output [N, D] (its 4 heads' contribution through
the output projection, bf16); the host sums the 4 partials per batch in fp32
and adds b_out.

Engine assignment per core (~156us, vs 222us for the phase-serial baseline):
  PE     all matmuls: QKV projection rounds, S^T (row-tiled head pairs),
         PV (with a ones column appended to V so the same matmul yields the
         softmax denominators), out-projection rounds (~120us busy)
  ACT    softmax exp only: exp(0.125*s + pad_bias) -> bf16 (~75us)
  DVE    PSUM evacuations (casts), causal band adds, denominator reciprocal
  GPSIMD denominator partition-broadcast ONLY (mixing gpsimd op types makes
         walrus swap the firmware library around every op, ~7us each)
  DMA    both queues (SP + ACT) for the input load; bf16 outputs

Schedule: attention units (head-pair, q-tile 512) run in sequence; a unit's
S^T->exp chain is ACT-bound, so projection/V/out-projection rounds are
injected between S^T matmuls as PE "filler" to keep the PE dense (HAM stays
at K=8/8).  Each unit's prereq rounds are HARD-emitted before the unit: the
Tile tracker is emission-ordered, so a read emitted before its writer gets no
dependency and reads garbage — filler pacing is a performance heuristic only.
PV matmuls of unit k are riffled into unit k+1's S^T stream with a small lag
so the in-order PE never drains on ScalarE.  Normalization runs one unit
behind: the denominator row [1,512] is folded onto 32 partitions with the
DVE 32x32 StreamTranspose, reciprocal'd 16-wide (the iterative divide is
8 cyc/element along the free dim), folded back, partition-broadcast on
GpSimd, and multiplied into ctx on DVE.  The [B,H,N,N] score tensor is
causally trimmed at 128-column granularity in S^T, exp, and PV.

Known dead ends (measured): fp8 anywhere gives 4-8e-2 rel err (gate 1e-2);
reciprocal_approx_fast (custom DVE uop) returns garbage/crashes the exec unit
under this runtime; exp(-ln(den)) on ScalarE forces an ACT table swap per
call (~38us total); SBUF->SBUF DMA cannot cross partitions, and DRAM-bounce
transposes race (DMA queue issue is async, Tile does not serialize the DRAM
RAW).
"""

import math
import os

import numpy as np

B, N, D, H = 2, 2048, 1024, 16
DK = D // H  # 64
NCORES = 8
HEADS_PER_CORE = 4
QTILE = 512
KBLK = 128
NEG = -30000.0
NEGB = -3750.0  # pad bias applied after the 0.125 scale inside exp
SCALE = 1.0 / math.sqrt(float(DK))  # 0.125
RIFFLE_LAG = 2  # PV chunks of unit k trail unit k+1's S^T by this many j's

# Set by run() when tracing is enabled (test.py reads this).
LAST_RESULTS = None


def _build_program(kb_max: int, jpad_min: int):
    import concourse.tile as tile
    from concourse import bacc, mybir

    F32 = mybir.dt.float32
    BF16 = mybir.dt.bfloat16
    EXP = mybir.ActivationFunctionType.Exp
    ADD = mybir.AluOpType.add

    nc = bacc.Bacc(None)

    xt_d = nc.dram_tensor("xt", [D, N], BF16, kind="ExternalInput")
    wq_d = nc.dram_tensor("wq", [D, 256], BF16, kind="ExternalInput")
    wk_d = nc.dram_tensor("wk", [D, 256], BF16, kind="ExternalInput")
    wv_d = nc.dram_tensor("wv", [D, 256], BF16, kind="ExternalInput")
    wout_d = nc.dram_tensor("wout", [256, D], BF16, kind="ExternalInput")
    padb_d = nc.dram_tensor("padbias", [128, 16], F32, kind="ExternalInput")
    trineg_d = nc.dram_tensor("trineg", [128, 896], BF16, kind="ExternalInput")
    out_d = nc.dram_tensor("out", [N, D], BF16, kind="ExternalOutput")

    NB = N // KBLK  # 16 key/row blocks
    NQT = N // QTILE  # 4 q tiles

    with tile.TileContext(nc) as tc:
        with (
            tc.tile_pool(name="w", bufs=1) as w_pool,
            tc.tile_pool(name="big", bufs=1) as big_pool,
            tc.tile_pool(name="work", bufs=3) as work_pool,
            tc.tile_pool(name="osb", bufs=3) as osb_pool,
            tc.tile_pool(name="xt", bufs=1) as xt_pool,
            tc.tile_pool(name="pt", bufs=26) as pt_pool,
            tc.tile_pool(name="ps_st", bufs=2, space="PSUM") as ps_st,
            tc.tile_pool(name="ps_b", bufs=2, space="PSUM") as ps_b,
            tc.tile_pool(name="ps_ctx", bufs=1, space="PSUM") as ps_ctx,
        ):
            # ---- input DMAs (weights first so projections can start early) --
            wq_t = w_pool.tile([128, 8, 256], BF16, tag="wq")
            wk_t = w_pool.tile([128, 8, 256], BF16, tag="wk")
            wv_t = w_pool.tile([128, 8, 256], BF16, tag="wv")
            wo_t = w_pool.tile([128, 2, D], BF16, tag="wo")
            padb_t = w_pool.tile([128, 16], F32, tag="padb")
            trineg_t = w_pool.tile([128, 896], BF16, tag="trineg")
            # Input DMAs ride BOTH hardware DMA queues (SP + Activation) —
            # a single queue is strictly serial at ~220GB/s and the load
            # gates the whole pipeline.  xt arrives as 32 column-chunked
            # tiles, q-tile-major, so the first projection rounds complete
            # after ~1MB instead of the full 4MB load.
            # Input DMAs ride BOTH hardware DMA queues (SP + Activation) —
            # a single queue is strictly serial at ~220GB/s and the load
            # gates the whole pipeline.  xt arrives as 32 column-chunked
            # tiles, q-tile-major, so the first projection rounds complete
            # after ~1MB instead of the full 4MB load.
            nc.sync.dma_start(wq_t[:], wq_d[:].rearrange("(e p) m -> p e m", p=128))
            nc.scalar.dma_start(wk_t[:], wk_d[:].rearrange("(e p) m -> p e m", p=128))
            xt = [[None] * NQT for _ in range(8)]
            for c in range(NQT):
                for e in range(8):
                    t = xt_pool.tile(
                        [128, 512], BF16, tag=f"xt{e}_{c}", name=f"xt{e}_{c}"
                    )
                    # ACT-queue DMAs only for the head-critical c=0 chunks;
                    # later chunks stay off the exp-stream queue
                    eng = nc.scalar if (c == 0 and e >= 4) else nc.sync
                    eng.dma_start(
                        t[:], xt_d[e * 128:(e + 1) * 128, c * 512:(c + 1) * 512]
                    )
                    xt[e][c] = t
                if c == 0:
                    nc.sync.dma_start(wv_t[:], wv_d[:].rearrange("(e p) m -> p e m", p=128))
                    nc.scalar.dma_start(trineg_t[:], trineg_d[:])
                    nc.scalar.dma_start(padb_t[:], padb_d[:])
            nc.sync.dma_start(wo_t[:], wout_d[:].rearrange("(c p) m -> p c m", p=128))

            # V' tile: [keys 128, key-block, head 4, 65]; col 64 <- ones so
            # P@V' also yields the softmax denominator on ctx row 64.
            v4 = big_pool.tile([128, kb_max, 4, 65], BF16, tag="v4")
            nc.gpsimd.memset(v4[:, :, :, 64:65], 1.0)

            # warm the ACT exp table during the DMA head so the ~2.7us
            # table load is off the critical path
            warm = work_pool.tile([1, 8], F32, tag="warm", name="warm")
            nc.vector.memset(warm[:], 1.0)
            nc.scalar.activation(warm[:], warm[:], EXP)

            qt_pair = [big_pool.tile([128, N], BF16, tag=f"qt{p}", name=f"qt{p}") for p in range(2)]
            kt_pair = [big_pool.tile([128, N], BF16, tag=f"kt{p}", name=f"kt{p}") for p in range(2)]
            ctx_pair = [big_pool.tile([128, N], BF16, tag=f"ctx{p}", name=f"ctx{p}") for p in range(2)]

            # ---- PE filler rounds (projections / V / out-projection) -------
            # Each round is ~1-4us of dense PE work ending in one DVE
            # evacuation; they are injected between attention steps to keep
            # the PE busy while ScalarE works through the exp chain.
            pe_ns = [0.0]  # emitted PE work (ns)
            act_ns = [0.0]  # emitted ACT work (ns)

            def qk_round(w_t, pair, nq, dst):
                ps = ps_b.tile([128, 512], F32, tag="b", name="b")
                for e in range(8):
                    nc.tensor.matmul(
                        ps[:],
                        w_t[:, e, pair * 128:(pair + 1) * 128],
                        xt[e][nq][:],
                        start=(e == 0),
                        stop=(e == 7),
                    )
                nc.vector.tensor_copy(dst[pair][:, nq * 512:(nq + 1) * 512], ps[:])
                pe_ns[0] += 8 * 512 / 2.4

            def v_round(nb):
                ps = ps_b.tile([128, 512], F32, tag="b", name="b")[:, 0:256]
                c, coff = divmod(nb, 4)
                for e in range(8):
                    nc.tensor.matmul(
                        ps[:],
                        xt[e][c][:, coff * 128:(coff + 1) * 128],
                        wv_t[:, e, :],
                        start=(e == 0),
                        stop=(e == 7),
                    )
                nc.vector.tensor_copy(
                    v4[:, nb, :, 0:64], ps[:].rearrange("p (h d) -> p h d", h=4)
                )
                pe_ns[0] += 8 * 256 / 2.4

            osb_tiles = {}

            def o_round(nb, fc):
                if fc == 0:
                    osb_tiles[nb] = osb_pool.tile([128, D], BF16, tag="osb", name="osb")
                osb = osb_tiles[nb]
                ps = ps_b.tile([128, 512], F32, tag="b", name="b")
                for pr2 in range(2):
                    nc.tensor.matmul(
                        ps[:],
                        ctx_pair[pr2][:, nb * 128:(nb + 1) * 128],
                        wo_t[:, pr2, fc * 512:(fc + 1) * 512],
                        start=(pr2 == 0),
                        stop=(pr2 == 1),
                    )
                nc.vector.tensor_copy(osb[:, fc * 512:(fc + 1) * 512], ps[:])
                if fc == 1:
                    nc.sync.dma_start(out_d[nb * 128:(nb + 1) * 128, :], osb[:])
                    del osb_tiles[nb]
                pe_ns[0] += 2 * 512 / 2.4

            rounds = {}
            for pair in range(2):
                for nq in range(NQT):
                    rounds[("q", pair, nq)] = (lambda p=pair, n=nq: qk_round(wq_t, p, n, qt_pair))
                    rounds[("k", pair, nq)] = (lambda p=pair, n=nq: qk_round(wk_t, p, n, kt_pair))
            for nb in range(kb_max):
                rounds[("v", nb)] = (lambda n=nb: v_round(n))
            for nb in range(NB):
                for fc in range(2):
                    rounds[("o", nb, fc)] = (lambda n=nb, f=fc: o_round(n, f))

            emitted = set()
            filler_q = []

            def emit_rid(rid):
                if rid in emitted:
                    return
                emitted.add(rid)
                rounds[rid]()

            def inject_fillers():
                # keep ~4us of emitted-but-unexecuted PE work beyond the ACT
                # frontier so the PE (the critical engine) never drains
                while filler_q and pe_ns[0] < act_ns[0] + 4000.0:
                    emit_rid(filler_q.pop(0))

            # prereqs of the first two units, emitted up front
            for rid in [("q", 0, 0), ("k", 0, 0), ("q", 1, 0), ("k", 1, 0),
                        ("v", 0), ("v", 1), ("v", 2), ("v", 3)]:
                emit_rid(rid)
            # remaining projection/V rounds become filler, ordered by the
            # deadline of the unit that first needs them
            for qt in range(1, NQT):
                for pair in range(2):
                    filler_q.append(("q", pair, qt))
                    filler_q.append(("k", pair, qt))
                for nb in range(4 * qt, min(4 * qt + 4, kb_max)):
                    filler_q.append(("v", nb))

            # ---- attention units ------------------------------------------
            def emit_normalize(pair, hh, qt, ctx_ps):
                # The DVE reciprocal is an iterative 8-cyc/element divide
                # streaming the FREE dim, and the denominator row is 512
                # elements on ONE partition (3.3us/call there).  Use the
                # DVE 32x32 StreamTranspose to fold the row onto 32
                # partitions, take the reciprocal 16-wide, and fold back
                # (bf16 on the way back: DVE 2x, ~0.4% rms).
                hp = slice(64 * hh, 64 * hh + 64)
                craw = work_pool.tile([65, 512], F32, tag="craw", name="craw")
                nc.vector.tensor_copy(craw[:], ctx_ps[0:65, :])
                tscat = work_pool.tile([32, 512], F32, tag="tscat", name="tscat")
                nc.vector.transpose(tscat[:], ctx_ps[64:96, :])
                rscat = work_pool.tile([32, 512], BF16, tag="rscat", name="rscat")
                with nc.allow_low_precision(
                    reason="bf16 softmax-denominator reciprocal: ~0.4% rms "
                    "on the normalize scale, inside the error budget"
                ):
                    nc.vector.reciprocal(
                        rscat[:].rearrange("p (b s) -> p b s", s=32)[:, :, 0],
                        tscat[:].rearrange("p (b s) -> p b s", s=32)[:, :, 0],
                    )
                rrow = work_pool.tile([32, 512], BF16, tag="rrow", name="rrow")
                nc.vector.transpose(rrow[:], rscat[:])
                act_ns[0] += 2 * (512 + 352) / 1.2
                rbr = work_pool.tile([64, 512], BF16, tag="rbr", name="rbr")
                # GpSimd runs ONLY partition_broadcast ops: mixing op types
                # makes walrus swap the gpsimd firmware library around every
                # op (~7us per swap)
                nc.gpsimd.partition_broadcast(rbr[:], rrow[0:1, :])
                nc.vector.tensor_mul(
                    ctx_pair[pair][hp, qt * 512:(qt + 1) * 512],
                    craw[0:64, :],
                    rbr[:],
                )

            last_unit = [False]  # force-drain fillers during the final unit

            def emit_st_exp(pair, qt, nchunks, prev):
                """S^T + mask + exp for both heads, with the previous unit's
                PV matmuls riffled in (lagged so the PE never stalls on the
                exp pipeline) and PE filler rounds injected to cover the
                ACT-bound stretch.  Returns PV descriptors."""
                if prev is None:
                    ppv = []
                else:
                    ppair, pqt, pn, ppv, pctx2 = prev

                def rif(k):
                    while ppv and ppv[0][0] <= k:
                        jj, ptt, poff = ppv.pop(0)
                        for hh in range(2):
                            nc.tensor.matmul(
                                pctx2[hh][0:65, poff:],
                                v4[:, jj, 2 * ppair + hh, :],
                                ptt[:, hh, poff:],
                                start=(jj == 0),
                                stop=(jj == pn - 1),
                                skip_group_check=True,
                            )
                        pe_ns[0] += 2 * (512 - poff) / 2.4

                pv = []
                for j in range(nchunks):
                    rif(j - RIFFLE_LAG)
                    inject_fillers()
                    if last_unit[0]:
                        # the PE FIFO can't reach past the last unit's
                        # exp-paced S^T stream; spread the remaining
                        # dep-ready fillers through it instead of after it
                        for _ in range(2):
                            if filler_q:
                                emit_rid(filler_q.pop(0))
                    d = j - 4 * qt
                    # exact-causal column trim (keep matmul N >= 128)
                    off = 128 * d if d >= 1 else 0
                    st_ps = ps_st.tile([128, 2, 512], F32, tag="blk", name="blk")
                    for hh in range(2):
                        hp = slice(64 * hh, 64 * hh + 64)
                        nc.tensor.matmul(
                            st_ps[:, hh, off:],
                            kt_pair[pair][hp, j * 128:(j + 1) * 128],
                            qt_pair[pair][hp, qt * 512 + off:(qt + 1) * 512],
                            start=True,
                            stop=True,
                        )
                    pe_ns[0] += (512 - off) / 2.4
                    if d >= 0:
                        # causal add -30000; with off = 128*d the masked
                        # triangle lies entirely in cols [off, off+128);
                        # one op covers both heads via a stride-0 broadcast
                        u0 = 384 - 128 * d + off
                        w = min(128, 512 - off)
                        nc.vector.tensor_tensor(
                            st_ps[:, :, off:off + w],
                            st_ps[:, :, off:off + w],
                            trineg_t[:, u0:u0 + w].unsqueeze(1).broadcast_to(
                                (128, 2, w)
                            ),
                            ADD,
                        )
                    pt_t = pt_pool.tile([128, 2, 512], BF16, tag="pt")
                    kw = {}
                    if j >= jpad_min:  # per-key pad bias (same for both heads)
                        kw["bias"] = padb_t[:, j:j + 1]
                    nc.scalar.activation(
                        pt_t[:, :, off:], st_ps[:, :, off:], EXP, scale=SCALE, **kw
                    )
                    act_ns[0] += (2 * (512 - off) + 352) / 1.2
                    pv.append((j, pt_t, off))
                rif(10 ** 9)
                return pv

            def emit_pv(pair, qt, nchunks, pv, ctx2):
                for j, pt_t, off in pv:
                    for hh in range(2):
                        nc.tensor.matmul(
                            ctx2[hh][0:65, off:],
                            v4[:, j, 2 * pair + hh, :],
                            pt_t[:, hh, off:],
                            start=(j == 0),
                            stop=(j == nchunks - 1),
                            skip_group_check=True,
                        )
                    pe_ns[0] += 2 * (512 - off) / 2.4

            units = [
                (pair, qt, min(4 * qt + 4, kb_max))
                for qt in range(NQT)
                for pair in range(2)
            ]
            done_norms = {q: 0 for q in range(NQT)}

            def emit_norm_unit(npair, nqt, nctx2):
                for hh in range(2):
                    emit_normalize(npair, hh, nqt, nctx2[hh])
                done_norms[nqt] += 1
                if done_norms[nqt] == 2:
                    # out-projection rounds for this q-tile become filler
                    for nb in range(4 * nqt, 4 * nqt + 4):
                        filler_q.append(("o", nb, 0))
                        filler_q.append(("o", nb, 1))

            prev_pv = None  # (pair, qt, nchunks, pv_descs, ctx2)
            for pair, qt, nchunks in units:
                # HARD-emit this unit's projection/V prereqs before any of
                # its S^T/PV instructions.  A read emitted before its writer
                # gets NO dependency from the Tile tracker (emission-ordered)
                # and would consume uninitialized SBUF; filler pacing alone
                # must never be trusted for correctness.
                emit_rid(("q", pair, qt))
                for nq in range(qt + 1):
                    emit_rid(("k", pair, nq))
                for nb in range(nchunks):
                    emit_rid(("v", nb))
                last_unit[0] = (pair, qt, nchunks) == units[-1]
                pv = emit_st_exp(pair, qt, nchunks, prev_pv)
                if prev_pv is not None:
                    ppair, pqt, pn, ppv, pctx2 = prev_pv
                    emit_norm_unit(ppair, pqt, pctx2)
                # [96, 512] so the normalize can StreamTranspose rows 64:96
                # (only 0:65 are written; same single PSUM bank either way)
                ctx2 = [
                    ps_ctx.tile([96, 512], F32, tag=f"ctx{hh}", name=f"ctx{hh}")
                    for hh in range(2)
                ]
                prev_pv = (pair, qt, nchunks, pv, ctx2)
            ppair, pqt, pn, ppv, pctx2 = prev_pv
            emit_pv(ppair, pqt, pn, ppv, pctx2)
            emit_norm_unit(ppair, pqt, pctx2)
            while filler_q:
                emit_rid(filler_q.pop(0))

    nc.compile()
    return nc


_PROGRAM_CACHE = {}


def kernel(x, attention_mask, W_Q, W_K, W_V, W_out, b_out):
    global LAST_RESULTS
    from concourse.bass_utils import run_bass_kernel_spmd

    x = np.ascontiguousarray(x, dtype=np.float32)
    attention_mask = np.asarray(attention_mask)
    lengths = attention_mask.astype(np.int64).sum(axis=1)
    kb_max = int(math.ceil(lengths.max() / KBLK))
    jpad_min = int(lengths.min() // KBLK)

    key = (kb_max, jpad_min)
    if key not in _PROGRAM_CACHE:
        _PROGRAM_CACHE[key] = _build_program(kb_max, jpad_min)
    nc = _PROGRAM_CACHE[key]

    # host-side input prep (matmul operands pre-cast to bf16)
    import ml_dtypes
    BF = ml_dtypes.bfloat16
    xT = [np.ascontiguousarray(x[b].T.astype(BF)) for b in range(B)]
    wqT = np.ascontiguousarray(np.asarray(W_Q, dtype=np.float32).T.astype(BF))
    wkT = np.ascontiguousarray(np.asarray(W_K, dtype=np.float32).T.astype(BF))
    wvT = np.ascontiguousarray(np.asarray(W_V, dtype=np.float32).T.astype(BF))
    woT = np.ascontiguousarray(np.asarray(W_out, dtype=np.float32).T.astype(BF))
    # padbias[p, j] = 0 if key j*128+p is real else NEGB
    padb = [
        np.ascontiguousarray(
            np.where(attention_mask[b].reshape(16, 128).T != 0, 0.0, NEGB)
        ).astype(np.float32)
        for b in range(B)
    ]
    # trineg[p, u] = NEG if u < p + 384 else 0; slice [384-128d : 896-128d]
    # gives the causal additive mask for a diagonal block with offset 128d.
    pp = np.arange(128)[:, None]
    uu = np.arange(896)[None, :]
    trineg = np.where(uu < pp + 384, NEG, 0.0).astype(BF)

    in_maps = []
    for c in range(NCORES):
        b, g = divmod(c, 4)
        sl = slice(g * 256, (g + 1) * 256)
        in_maps.append(
            {
                "xt": xT[b],
                "wq": np.ascontiguousarray(wqT[:, sl]),
                "wk": np.ascontiguousarray(wkT[:, sl]),
                "wv": np.ascontiguousarray(wvT[:, sl]),
                "wout": np.ascontiguousarray(woT[sl, :]),
                "padbias": padb[b],
                "trineg": trineg,
            }
        )

    trace = bool(int(os.environ.get("KERNEL_TRACE", "0")))
    ncores_run = int(os.environ.get("KERNEL_NCORES", str(NCORES)))
    res = run_bass_kernel_spmd(
        nc,
        in_maps[:ncores_run],
        core_ids=list(range(ncores_run)),
        trace=trace,
        trace_cores=list(range(ncores_run)) if trace else None,
    )
    LAST_RESULTS = res

    out = np.zeros((B, N, D), dtype=np.float32)
    for c in range(len(res.results)):
        out[c // 4] += np.asarray(res.results[c]["out"], dtype=np.float32)
    out += np.asarray(b_out, dtype=np.float32)[None, None, :]
    return out
